# revision 1
# baseline (speedup 1.0000x reference)
"""BatchTopK (training-mode) Trainium2 kernel.

Reference semantics (hardcoded for x: [4096, 24576] f32):
    total_k  = 64 * 4096 = 262144
    thr      = 262144-th largest value of x (min of global top-k)
    out      = relu(x) * (x >= thr)

Strategy (8 NeuronCores, data-parallel over rows, 512 rows/core):
  Phase 1 (device): each core streams its 48 MiB shard once and emits the
    top-8 values of every 384-element window (InstMax on VectorE). Any
    element of the global top-262144 set is in some window's top-8 unless
    a 384-window holds >8 such elements (zero windows do for the actual
    key(0) input, ~0.3 expected misses for a fresh randn draw, and a miss
    only shifts the selected rank by ~1, moving the threshold by ~1e-6)
    -> candidate set of 8/384 of the data, exact w.h.p.
  Host: exact rank selection (np.partition) over the ~2.1M candidates ->
    global threshold, bit-exact.
  Phase 2 (device): out = (x >= thr) * x per tile (one VectorE
    scalar_tensor_tensor, valid since thr > 0; host-verified with exact
    numpy fallback otherwise). Pure stream kernel at HBM line rate.
"""

import sys

sys.path.insert(0, "/opt/trn_rl_repo")

import numpy as np

import concourse.bass as bass
import concourse.mybir as mybir
from concourse import tile
from concourse.bass_utils import run_bass_kernel_spmd

# Problem geometry (hardcoded per spec)
R, C = 4096, 24576
K_TOTAL = 64 * R
N_CORES = 8
RS = R // N_CORES            # rows per core shard = 512
P = 128                      # SBUF partitions
FREE = RS * C // P           # free elems per partition = 98304

# Phase-1 tiling. All chunks are multiples of W. (Tapered final chunks
# were tried and measured identical — run-to-run HBM contention noise
# dominates the ~10us tail they save.)
W = 384                      # top-8 extraction window
CHUNKS1 = [12288] * 8                            # sums to FREE
CAND_PER_P = (FREE // W) * 8  # 2048 candidate slots per partition

# Phase-2 tiling
CHUNKS2 = [8192] * 12                            # sums to FREE

FP32 = mybir.dt.float32

_programs = {}
last_exec_ns = {}


def _split_excess_waits(nc: bass.Bass) -> None:
    """walrus on this toolchain rejects instructions whose embedded SyncWait
    list exceeds the ISA encoding: DMA queue instructions take 1 wait,
    engine instructions take 2. Tile can emit more. Hoist the excess into
    standalone InstEventSemaphore waits on the same engine immediately
    before the instruction — identical semantics (the sequencer executes
    the waits right before the instruction either way)."""
    dma_types = (mybir.InstDMACopy, mybir.InstDMA, mybir.InstTensorLoad,
                 mybir.InstTensorSave, mybir.InstLoad, mybir.InstSave)
    for f in nc.m.functions:
        for b in f.blocks:
            new_insts = []
            for inst in b.instructions:
                si = getattr(inst, "sync_info", None)
                waits = list(si.on_wait) if si is not None and si.on_wait else []
                cap = 1
                if len(waits) > cap:
                    keep, excess = waits[:cap], waits[cap:]
                    for w in excess:
                        ev = mybir.InstEventSemaphore(
                            name=f"I-wsplit-{nc.next_id()}",
                            ins=[], outs=[],
                            sync_info=mybir.SyncInfo(on_wait=[w], on_update=[]),
                            bass_nofuse=True,
                        )
                        ev.engine = inst.engine
                        new_insts.append(ev)
                    inst.sync_info = mybir.SyncInfo(
                        on_wait=keep, on_update=list(si.on_update or []))
                new_insts.append(inst)
            b.instructions[:] = new_insts


def _build_phase1() -> bass.Bass:
    nc = bass.Bass("TRN2", target_bir_lowering=False, debug=False,
                   num_devices=N_CORES)
    x = nc.dram_tensor("x", [P, FREE], FP32, kind="ExternalInput")
    cand = nc.dram_tensor("cand", [P, CAND_PER_P], FP32, kind="ExternalOutput")
    xv = x.ap()
    with tile.TileContext(nc) as tc:
        with (
            tc.tile_pool(name="io", bufs=3) as pool,
            tc.tile_pool(name="cd", bufs=len(CHUNKS1)) as cpool,
        ):
            off = coff = 0
            for ch in CHUNKS1:
                nw = ch // W
                cpp = nw * 8
                xt = pool.tile([P, ch], FP32)
                nc.sync.dma_start(out=xt[:], in_=xv[:, off:off + ch])
                cand_t = cpool.tile([P, cpp], FP32)
                for w in range(nw):
                    nc.vector.max(cand_t[:, w * 8:(w + 1) * 8],
                                  xt[:, w * W:(w + 1) * W])
                nc.sync.dma_start(out=cand.ap()[:, coff:coff + cpp],
                                  in_=cand_t[:])
                off += ch
                coff += cpp
    return nc


def _build_phase2() -> bass.Bass:
    nc = bass.Bass("TRN2", target_bir_lowering=False, debug=False,
                   num_devices=N_CORES)
    x = nc.dram_tensor("x", [P, FREE], FP32, kind="ExternalInput")
    thr = nc.dram_tensor("thr", [P, 1], FP32, kind="ExternalInput")
    out = nc.dram_tensor("out", [P, FREE], FP32, kind="ExternalOutput")
    xv, ov = x.ap(), out.ap()
    with tile.TileContext(nc) as tc:
        with (
            tc.tile_pool(name="io", bufs=4) as xpool,
            tc.tile_pool(name="t", bufs=1) as tpool,
        ):
            thr_t = tpool.tile([P, 1], FP32)
            nc.sync.dma_start(out=thr_t[:], in_=thr.ap())
            off = 0
            for ch in CHUNKS2:
                sl = slice(off, off + ch)
                xt = xpool.tile([P, ch], FP32)
                nc.sync.dma_start(out=xt[:], in_=xv[:, sl])
                # xt = (xt >= thr) * xt  (== relu(x)*(x >= thr) when thr > 0;
                # host falls back to numpy for thr <= 0)
                nc.vector.scalar_tensor_tensor(
                    out=xt[:], in0=xt[:], scalar=thr_t[:, 0:1], in1=xt[:],
                    op0=mybir.AluOpType.is_ge, op1=mybir.AluOpType.mult,
                )
                nc.sync.dma_start(out=ov[:, sl], in_=xt[:])
                off += ch
    return nc


def _get_program(name):
    if name not in _programs:
        nc = _build_phase1() if name == "p1" else _build_phase2()
        _split_excess_waits(nc)
        _programs[name] = nc
    return _programs[name]


def kernel(x: np.ndarray, trace: bool = False) -> np.ndarray:
    x = np.asarray(x)
    assert x.shape == (R, C), x.shape
    if x.dtype != np.float32:
        x = x.astype(np.float32)
    core_ids = list(range(N_CORES))
    shards = [np.ascontiguousarray(x[c * RS:(c + 1) * RS].reshape(P, FREE))
              for c in range(N_CORES)]

    # Phase 1: candidate extraction
    p1 = _get_program("p1")
    res1 = run_bass_kernel_spmd(p1, [{"x": s} for s in shards], core_ids,
                                trace=trace)
    last_exec_ns["p1"] = res1.exec_time_ns
    cands = np.concatenate([r["cand"].ravel() for r in res1.results])

    # Host: exact global rank selection over candidates
    idx = cands.size - K_TOTAL
    thr = np.partition(cands, idx)[idx]

    if not thr > 0:
        # Device phase 2 assumes thr > 0 (true for any remotely
        # normal-like input: top 0.26% of values). Exact host fallback.
        return (np.maximum(x, 0.0) * (x >= thr)).astype(np.float32)

    # Phase 2: masking pass
    p2 = _get_program("p2")
    thr_arr = np.full((P, 1), thr, dtype=np.float32)
    res2 = run_bass_kernel_spmd(
        p2, [{"x": s, "thr": thr_arr} for s in shards], core_ids, trace=trace)
    last_exec_ns["p2"] = res2.exec_time_ns

    return np.concatenate(
        [r["out"].reshape(RS, C) for r in res2.results], axis=0)



# revision 2
# speedup vs baseline: 1.0515x; 1.0515x over previous
"""BatchTopK (training-mode) Trainium2 kernel.

Reference semantics (hardcoded for x: [4096, 24576] f32):
    total_k  = 64 * 4096 = 262144
    thr      = 262144-th largest value of x (min of global top-k)
    out      = relu(x) * (x >= thr)

Strategy (8 NeuronCores, data-parallel over rows, 512 rows/core):
  Phase 1 (device): each core streams its 48 MiB shard once and emits the
    top-8 values of every 384-element window (InstMax on VectorE). Any
    element of the global top-262144 set is in some window's top-8 unless
    a 384-window holds >8 such elements (zero windows do for the actual
    key(0) input, ~0.3 expected misses for a fresh randn draw, and a miss
    only shifts the selected rank by ~1, moving the threshold by ~1e-6)
    -> candidate set of 8/384 of the data, exact w.h.p.
  Host: exact rank selection (np.partition) over the ~2.1M candidates ->
    global threshold, bit-exact.
  Phase 2 (device): out = (x >= thr) * x per tile (one VectorE
    scalar_tensor_tensor, valid since thr > 0; host-verified with exact
    numpy fallback otherwise). Pure stream kernel at HBM line rate.
"""

import sys

sys.path.insert(0, "/opt/trn_rl_repo")

import numpy as np

import concourse.bass as bass
import concourse.mybir as mybir
from concourse import tile
from concourse.bass_utils import run_bass_kernel_spmd

# Problem geometry (hardcoded per spec)
R, C = 4096, 24576
K_TOTAL = 64 * R
N_CORES = 8
RS = R // N_CORES            # rows per core shard = 512
P = 128                      # SBUF partitions
FREE = RS * C // P           # free elems per partition = 98304

# Phase-1 tiling. All chunks are multiples of W. Head+tail tapering
# measured faster (159us vs 172us interleaved A/B): the first InstMax can
# start after a 4.4us chunk instead of 17.5us, and the final chunk's
# InstMax tail shrinks likewise. Mid-stream chunks stay large because
# per-DMA queue+semaphore overhead (~0.75us/instr) dominates fine tiling.
W = 384                      # top-8 extraction window
CHUNKS1 = [3072, 3072, 6144, 12288, 12288, 12288, 12288, 12288, 12288, 6144, 3072, 3072]  # tapered ramp/tail; sums to FREE
CAND_PER_P = (FREE // W) * 8  # 2048 candidate slots per partition

# Phase-2 tiling
CHUNKS2 = [8192] * 12                            # sums to FREE

FP32 = mybir.dt.float32

_programs = {}
last_exec_ns = {}


def _split_excess_waits(nc: bass.Bass) -> None:
    """walrus on this toolchain rejects instructions whose embedded SyncWait
    list exceeds the ISA encoding: DMA queue instructions take 1 wait,
    engine instructions take 2. Tile can emit more. Hoist the excess into
    standalone InstEventSemaphore waits on the same engine immediately
    before the instruction — identical semantics (the sequencer executes
    the waits right before the instruction either way)."""
    dma_types = (mybir.InstDMACopy, mybir.InstDMA, mybir.InstTensorLoad,
                 mybir.InstTensorSave, mybir.InstLoad, mybir.InstSave)
    for f in nc.m.functions:
        for b in f.blocks:
            new_insts = []
            for inst in b.instructions:
                si = getattr(inst, "sync_info", None)
                waits = list(si.on_wait) if si is not None and si.on_wait else []
                cap = 1
                if len(waits) > cap:
                    keep, excess = waits[:cap], waits[cap:]
                    for w in excess:
                        ev = mybir.InstEventSemaphore(
                            name=f"I-wsplit-{nc.next_id()}",
                            ins=[], outs=[],
                            sync_info=mybir.SyncInfo(on_wait=[w], on_update=[]),
                            bass_nofuse=True,
                        )
                        ev.engine = inst.engine
                        new_insts.append(ev)
                    inst.sync_info = mybir.SyncInfo(
                        on_wait=keep, on_update=list(si.on_update or []))
                new_insts.append(inst)
            b.instructions[:] = new_insts


def _build_phase1() -> bass.Bass:
    nc = bass.Bass("TRN2", target_bir_lowering=False, debug=False,
                   num_devices=N_CORES)
    x = nc.dram_tensor("x", [P, FREE], FP32, kind="ExternalInput")
    cand = nc.dram_tensor("cand", [P, CAND_PER_P], FP32, kind="ExternalOutput")
    xv = x.ap()
    with tile.TileContext(nc) as tc:
        with (
            tc.tile_pool(name="io", bufs=4) as pool,
            tc.tile_pool(name="cd", bufs=8) as cpool,
        ):
            off = coff = 0
            for ch in CHUNKS1:
                nw = ch // W
                cpp = nw * 8
                xt = pool.tile([P, ch], FP32)
                nc.sync.dma_start(out=xt[:], in_=xv[:, off:off + ch])
                cand_t = cpool.tile([P, cpp], FP32)
                for w in range(nw):
                    nc.vector.max(cand_t[:, w * 8:(w + 1) * 8],
                                  xt[:, w * W:(w + 1) * W])
                nc.sync.dma_start(out=cand.ap()[:, coff:coff + cpp],
                                  in_=cand_t[:])
                off += ch
                coff += cpp
    return nc


def _build_phase2() -> bass.Bass:
    nc = bass.Bass("TRN2", target_bir_lowering=False, debug=False,
                   num_devices=N_CORES)
    x = nc.dram_tensor("x", [P, FREE], FP32, kind="ExternalInput")
    thr = nc.dram_tensor("thr", [P, 1], FP32, kind="ExternalInput")
    out = nc.dram_tensor("out", [P, FREE], FP32, kind="ExternalOutput")
    xv, ov = x.ap(), out.ap()
    with tile.TileContext(nc) as tc:
        with (
            tc.tile_pool(name="io", bufs=4) as xpool,
            tc.tile_pool(name="t", bufs=1) as tpool,
        ):
            thr_t = tpool.tile([P, 1], FP32)
            nc.sync.dma_start(out=thr_t[:], in_=thr.ap())
            off = 0
            for ch in CHUNKS2:
                sl = slice(off, off + ch)
                xt = xpool.tile([P, ch], FP32)
                nc.sync.dma_start(out=xt[:], in_=xv[:, sl])
                # xt = (xt >= thr) * xt  (== relu(x)*(x >= thr) when thr > 0;
                # host falls back to numpy for thr <= 0)
                nc.vector.scalar_tensor_tensor(
                    out=xt[:], in0=xt[:], scalar=thr_t[:, 0:1], in1=xt[:],
                    op0=mybir.AluOpType.is_ge, op1=mybir.AluOpType.mult,
                )
                nc.sync.dma_start(out=ov[:, sl], in_=xt[:])
                off += ch
    return nc


def _get_program(name):
    if name not in _programs:
        nc = _build_phase1() if name == "p1" else _build_phase2()
        _split_excess_waits(nc)
        _programs[name] = nc
    return _programs[name]


def kernel(x: np.ndarray, trace: bool = False) -> np.ndarray:
    x = np.asarray(x)
    assert x.shape == (R, C), x.shape
    if x.dtype != np.float32:
        x = x.astype(np.float32)
    core_ids = list(range(N_CORES))
    shards = [np.ascontiguousarray(x[c * RS:(c + 1) * RS].reshape(P, FREE))
              for c in range(N_CORES)]

    # Phase 1: candidate extraction
    p1 = _get_program("p1")
    res1 = run_bass_kernel_spmd(p1, [{"x": s} for s in shards], core_ids,
                                trace=trace)
    last_exec_ns["p1"] = res1.exec_time_ns
    cands = np.concatenate([r["cand"].ravel() for r in res1.results])

    # Host: exact global rank selection over candidates
    idx = cands.size - K_TOTAL
    thr = np.partition(cands, idx)[idx]

    if not thr > 0:
        # Device phase 2 assumes thr > 0 (true for any remotely
        # normal-like input: top 0.26% of values). Exact host fallback.
        return (np.maximum(x, 0.0) * (x >= thr)).astype(np.float32)

    # Phase 2: masking pass
    p2 = _get_program("p2")
    thr_arr = np.full((P, 1), thr, dtype=np.float32)
    res2 = run_bass_kernel_spmd(
        p2, [{"x": s, "thr": thr_arr} for s in shards], core_ids, trace=trace)
    last_exec_ns["p2"] = res2.exec_time_ns

    return np.concatenate(
        [r["out"].reshape(RS, C) for r in res2.results], axis=0)



# revision 3
# speedup vs baseline: 1.0982x; 1.0444x over previous
"""BatchTopK (training-mode) Trainium2 kernel.

Reference semantics (hardcoded for x: [4096, 24576] f32):
    total_k  = 64 * 4096 = 262144
    thr      = 262144-th largest value of x (min of global top-k)
    out      = relu(x) * (x >= thr)

Strategy (8 NeuronCores, data-parallel over rows, 512 rows/core):
  Phase 1 (device): each core streams its 48 MiB shard once and emits the
    top-8 values of every 384-element window (InstMax on VectorE). Any
    element of the global top-262144 set is in some window's top-8 unless
    a 384-window holds >8 such elements (zero windows do for the actual
    key(0) input, ~0.3 expected misses for a fresh randn draw, and a miss
    only shifts the selected rank by ~1, moving the threshold by ~1e-6)
    -> candidate set of 8/384 of the data, exact w.h.p.
  Host: exact rank selection (np.partition) over the ~2.1M candidates ->
    global threshold, bit-exact.
  Phase 2 (device): out = (x >= thr) * x per tile (one VectorE
    scalar_tensor_tensor, valid since thr > 0; host-verified with exact
    numpy fallback otherwise). Pure stream kernel at HBM line rate.
"""

import sys

sys.path.insert(0, "/opt/trn_rl_repo")

import numpy as np

import concourse.bass as bass
import concourse.mybir as mybir
from concourse import tile
from concourse.bass_utils import run_bass_kernel_spmd

# Problem geometry (hardcoded per spec)
R, C = 4096, 24576
K_TOTAL = 64 * R
N_CORES = 8
RS = R // N_CORES            # rows per core shard = 512
P = 128                      # SBUF partitions
FREE = RS * C // P           # free elems per partition = 98304

# Phase-1 tiling. All chunks are multiples of W. Head+tail tapering
# measured faster (159us vs 172us interleaved A/B): the first InstMax can
# start after a 4.4us chunk instead of 17.5us, and the final chunk's
# InstMax tail shrinks likewise. Mid-stream chunks stay large because
# per-DMA queue+semaphore overhead (~0.75us/instr) dominates fine tiling.
W = 384                      # top-8 extraction window
CHUNKS1 = [3072, 3072, 3072, 3072, 6144, 6144, 12288, 12288, 12288, 12288, 12288, 6144, 3072, 3072]  # tapered ramp/tail; sums to FREE
CAND_PER_P = (FREE // W) * 8  # 2048 candidate slots per partition

# Phase-2 tiling
CHUNKS2 = [8192] * 12                            # sums to FREE

FP32 = mybir.dt.float32

_programs = {}
last_exec_ns = {}


def _split_excess_waits(nc: bass.Bass) -> None:
    """walrus on this toolchain rejects instructions whose embedded SyncWait
    list exceeds the ISA encoding: DMA queue instructions take 1 wait,
    engine instructions take 2. Tile can emit more. Hoist the excess into
    standalone InstEventSemaphore waits on the same engine immediately
    before the instruction — identical semantics (the sequencer executes
    the waits right before the instruction either way)."""
    dma_types = (mybir.InstDMACopy, mybir.InstDMA, mybir.InstTensorLoad,
                 mybir.InstTensorSave, mybir.InstLoad, mybir.InstSave)
    for f in nc.m.functions:
        for b in f.blocks:
            new_insts = []
            for inst in b.instructions:
                si = getattr(inst, "sync_info", None)
                waits = list(si.on_wait) if si is not None and si.on_wait else []
                cap = 1
                if len(waits) > cap:
                    keep, excess = waits[:cap], waits[cap:]
                    for w in excess:
                        ev = mybir.InstEventSemaphore(
                            name=f"I-wsplit-{nc.next_id()}",
                            ins=[], outs=[],
                            sync_info=mybir.SyncInfo(on_wait=[w], on_update=[]),
                            bass_nofuse=True,
                        )
                        ev.engine = inst.engine
                        new_insts.append(ev)
                    inst.sync_info = mybir.SyncInfo(
                        on_wait=keep, on_update=list(si.on_update or []))
                new_insts.append(inst)
            b.instructions[:] = new_insts


def _build_phase1() -> bass.Bass:
    nc = bass.Bass("TRN2", target_bir_lowering=False, debug=False,
                   num_devices=N_CORES)
    x = nc.dram_tensor("x", [P, FREE], FP32, kind="ExternalInput")
    cand = nc.dram_tensor("cand", [P, CAND_PER_P], FP32, kind="ExternalOutput")
    xv = x.ap()
    with tile.TileContext(nc) as tc:
        with (
            tc.tile_pool(name="io", bufs=4) as pool,
            tc.tile_pool(name="cd", bufs=1) as cpool,
        ):
            # One persistent candidate tile; every InstMax writes its own
            # 8-col slice and a single DMA drains it at the end. The
            # per-chunk cand-out DMAs it replaces straggled ~13us after the
            # input stream finished (sem chains between 12 tiny DMAs).
            cand_t = cpool.tile([P, CAND_PER_P], FP32, name="cand_t")
            off = coff = 0
            for ch in CHUNKS1:
                nw = ch // W
                xt = pool.tile([P, ch], FP32, name="xt")
                nc.sync.dma_start(out=xt[:], in_=xv[:, off:off + ch])
                for w in range(nw):
                    nc.vector.max(cand_t[:, coff + w * 8:coff + (w + 1) * 8],
                                  xt[:, w * W:(w + 1) * W])
                off += ch
                coff += nw * 8
            nc.sync.dma_start(out=cand.ap()[:], in_=cand_t[:])
    return nc


def _build_phase2() -> bass.Bass:
    nc = bass.Bass("TRN2", target_bir_lowering=False, debug=False,
                   num_devices=N_CORES)
    x = nc.dram_tensor("x", [P, FREE], FP32, kind="ExternalInput")
    thr = nc.dram_tensor("thr", [P, 1], FP32, kind="ExternalInput")
    out = nc.dram_tensor("out", [P, FREE], FP32, kind="ExternalOutput")
    xv, ov = x.ap(), out.ap()
    with tile.TileContext(nc) as tc:
        with (
            tc.tile_pool(name="io", bufs=4) as xpool,
            tc.tile_pool(name="t", bufs=1) as tpool,
        ):
            thr_t = tpool.tile([P, 1], FP32)
            nc.sync.dma_start(out=thr_t[:], in_=thr.ap())
            off = 0
            for ch in CHUNKS2:
                sl = slice(off, off + ch)
                xt = xpool.tile([P, ch], FP32)
                nc.sync.dma_start(out=xt[:], in_=xv[:, sl])
                # xt = (xt >= thr) * xt  (== relu(x)*(x >= thr) when thr > 0;
                # host falls back to numpy for thr <= 0)
                nc.vector.scalar_tensor_tensor(
                    out=xt[:], in0=xt[:], scalar=thr_t[:, 0:1], in1=xt[:],
                    op0=mybir.AluOpType.is_ge, op1=mybir.AluOpType.mult,
                )
                nc.sync.dma_start(out=ov[:, sl], in_=xt[:])
                off += ch
    return nc


def _get_program(name):
    if name not in _programs:
        nc = _build_phase1() if name == "p1" else _build_phase2()
        _split_excess_waits(nc)
        _programs[name] = nc
    return _programs[name]


def kernel(x: np.ndarray, trace: bool = False) -> np.ndarray:
    x = np.asarray(x)
    assert x.shape == (R, C), x.shape
    if x.dtype != np.float32:
        x = x.astype(np.float32)
    core_ids = list(range(N_CORES))
    shards = [np.ascontiguousarray(x[c * RS:(c + 1) * RS].reshape(P, FREE))
              for c in range(N_CORES)]

    # Phase 1: candidate extraction
    p1 = _get_program("p1")
    res1 = run_bass_kernel_spmd(p1, [{"x": s} for s in shards], core_ids,
                                trace=trace)
    last_exec_ns["p1"] = res1.exec_time_ns
    cands = np.concatenate([r["cand"].ravel() for r in res1.results])

    # Host: exact global rank selection over candidates
    idx = cands.size - K_TOTAL
    thr = np.partition(cands, idx)[idx]

    if not thr > 0:
        # Device phase 2 assumes thr > 0 (true for any remotely
        # normal-like input: top 0.26% of values). Exact host fallback.
        return (np.maximum(x, 0.0) * (x >= thr)).astype(np.float32)

    # Phase 2: masking pass
    p2 = _get_program("p2")
    thr_arr = np.full((P, 1), thr, dtype=np.float32)
    res2 = run_bass_kernel_spmd(
        p2, [{"x": s, "thr": thr_arr} for s in shards], core_ids, trace=trace)
    last_exec_ns["p2"] = res2.exec_time_ns

    return np.concatenate(
        [r["out"].reshape(RS, C) for r in res2.results], axis=0)

